# revision 41
# baseline (speedup 1.0000x reference)
"""Trainium2 Bass kernel for nn_CPDFP_25701084299789 (pooling).

Reference math (B=64, C=256, H=W=32), per branch x in {x1, x2}:
    center  = x[:, :, 16, 16]                               (B, C)
    dot     = sum_c(x * center) / C                         (B, 1, H, W)
    attn    = sigmoid(conv_w @ concat([x, dot], ch))        (B, C, H, W)
    pool    = sum_hw(x * attn) / (sum_hw(attn) + 1e-8)      (B, C)
output = pool(x1) + pool(x2)   # the caLayer softmax is over a size-1
                               # axis == 1.0, so it contributes nothing.

Key algebraic simplification: the concat+1x1-conv is a rank-1 weight
update, Y_b = (W[:, :C] + w_last (outer) center_b / C) @ X_b, so no
extra matmuls for the `dot` channel are needed.

Sharding: data-parallel over batch across 8 cores; each core handles
8 batch items x 2 branches = 16 independent (256, 1024) items.
Params (conv_w-derived) replicated.
"""

import os
import threading
from contextlib import ExitStack

import numpy as np

import concourse.bacc as bacc
import concourse.mybir as mybir
import concourse.tile as tile
from concourse.bass_utils import run_bass_kernel_spmd

N_CORES = 8
B, C, HW = 64, 256, 1024          # batch, channels, H*W
B_LOC = B // N_CORES              # batch items per core
ITEMS = 2 * B_LOC                 # branch-items per core (x1 + x2)
CENTER = 16 * 32 + 16             # flat index of (h//2, w//2)
FP = mybir.dt.float32
FPR = mybir.dt.float32r           # same bits; PE runs 4x faster than fp32
BF = mybir.dt.bfloat16

# bf16 x-path: halves HBM traffic at the cost of ~bf16-level accuracy
# (rel err ~2e-3 vs f32r's ~1e-4) for only a ~10% modeled speedup (ACT/DVE
# see no bf16 gain on this op mix). Off by default.
USE_BF16 = os.environ.get("KERNEL_BF16", "0") == "1"
XDT = BF if USE_BF16 else FPR

_build_lock = threading.Lock()
_cached_nc = None


def _build():
    nc = bacc.Bacc()

    xdram_dt = BF if USE_BF16 else FP
    x1 = nc.dram_tensor("x1", [B_LOC, C, HW], xdram_dt, kind="ExternalInput")
    x2 = nc.dram_tensor("x2", [B_LOC, C, HW], xdram_dt, kind="ExternalInput")
    # wc[c, kh, o] = conv_w[o, kh*128 + c] for kh<2 (transposed lhsT layout);
    # wc[p, 2, o] = conv_w[o, C] / C (broadcast across partitions).
    wc = nc.dram_tensor("wc", [128, 3, C], FP, kind="ExternalInput")
    # rs[o_part, 0, 2*itm + m] = r = sum_hw(x*attn); rs[o_part, 1, ...] = s =
    # sum_hw(attn). Cols >= 2*ITEMS are spare partial-accumulator columns
    # (zeroed; _postprocess adds them unconditionally).
    NCOL = 2 * ITEMS + 6
    rs_out = nc.dram_tensor("rs", [128, 2, NCOL], FP, kind="ExternalOutput")

    with tile.TileContext(nc) as tc, ExitStack() as ctx:
        singles = ctx.enter_context(tc.tile_pool(name="singles", bufs=1))
        xpool = ctx.enter_context(tc.tile_pool(name="xp", bufs=10))
        wpool = ctx.enter_context(tc.tile_pool(name="wp", bufs=6))
        apool = ctx.enter_context(tc.tile_pool(name="ap", bufs=6))
        psum = ctx.enter_context(tc.tile_pool(name="ps", bufs=4, space="PSUM"))

        wc_sb = singles.tile([128, 3, C], FP)
        nc.sync.dma_start(out=wc_sb, in_=wc[:, :, :])
        rs_sb = singles.tile([128, 2, NCOL], FP)
        r_sb = rs_sb[:, 0, :]
        s_sb = rs_sb[:, 1, :]
        nc.vector.memset(rs_sb[:, :, 2 * ITEMS:], 0.0)
        # Absorb the weight-DMA wait into the consuming engines' vector
        # clocks up front, so per-item instructions don't each carry an
        # extra sync wait (walrus rejects ops with too many waits).
        absorb = singles.tile([128, 2], FP)
        nc.vector.tensor_copy(out=absorb[:, 0:1], in_=wc_sb[:, 0, 0:1])
        nc.gpsimd.tensor_copy(out=absorb[:, 1:2], in_=wc_sb[:, 0, 0:1])

        ADT = XDT if USE_BF16 else FP

        def make_weff(cen_aps):
            # weff[c, o] = wt[c, o] + x[c, center] * wlast[o]/C, on GpSimd
            # (Pool) — otherwise idle — keeping DVE for the r-reduce. Pool
            # rejects the fused STT op, so two steps.
            weffs = []
            for kh in range(2):
                if USE_BF16:
                    # tensor_scalar requires an fp32 scalar; upconvert first
                    cen = wpool.tile([128, 1], FP, tag="cen")
                    nc.gpsimd.tensor_copy(out=cen, in_=cen_aps[kh])
                    cen_ap = cen
                else:
                    cen_ap = cen_aps[kh].bitcast(FP)
                delta = wpool.tile([128, C], FP, tag="delta")
                nc.gpsimd.tensor_scalar_mul(delta, wc_sb[:, 2, :], cen_ap)
                weff = wpool.tile([128, C], XDT, tag="weff")
                nc.gpsimd.tensor_tensor(
                    out=weff, in0=delta, in1=wc_sb[:, kh, :],
                    op=mybir.AluOpType.add,
                )
                weffs.append(weff)
            return weffs

        def sig_and_reduce(y_ap, x_ap, a_ap, prod_ap, rcol):
            nc.scalar.activation(
                out=a_ap,
                in_=y_ap,
                func=mybir.ActivationFunctionType.Sigmoid,
                accum_out=s_sb[:, rcol:rcol + 1],
            )
            # Fused multiply + free-axis reduce: r = sum_hw(a * x).
            # (InstTensorTensorReduce miscompiles on this stack; the
            # TensorScalarPtr form with accum_out works.)
            nc.vector.scalar_tensor_tensor(
                out=prod_ap,
                in0=a_ap,
                scalar=1.0,
                in1=x_ap,
                op0=mybir.AluOpType.bypass,
                op1=mybir.AluOpType.mult,
                accum_out=r_sb[:, rcol:rcol + 1],
            )

        for itm in range(ITEMS):
            src = x1 if itm < B_LOC else x2
            bi = itm % B_LOC

            xs = []
            for kh in range(2):
                xt = xpool.tile([128, HW], XDT, tag="x")
                din = src[bi, kh * 128:(kh + 1) * 128, :]
                nc.sync.dma_start(out=xt, in_=din if USE_BF16 else din.bitcast(FPR))
                xs.append(xt)

            weffs = make_weff([x[:, CENTER:CENTER + 1] for x in xs])

            for m in range(2):  # output-channel halves
                y = psum.tile([128, HW], FP, tag="y")
                for n, kh in [(0, 0), (1, 0), (0, 1), (1, 1)]:  # kh-outer: LDW reuse
                    nc.tensor.matmul(
                        out=y[:, n * 512:(n + 1) * 512],
                        lhsT=weffs[kh][:, m * 128:(m + 1) * 128],
                        rhs=xs[kh][:, n * 512:(n + 1) * 512],
                        start=(kh == 0),
                        stop=(kh == 1),
                    )
                a = apool.tile([128, HW], ADT, tag="a")
                prod = apool.tile([128, HW], ADT, tag="prod")
                sig_and_reduce(y, xs[m], a, prod, 2 * itm + m)

        nc.sync.dma_start(out=rs_out[:, :, :], in_=rs_sb)

    nc.finalize()
    return nc


def _get_nc():
    global _cached_nc
    with _build_lock:
        if _cached_nc is None:
            _cached_nc = _build()
    return _cached_nc


def _make_in_maps(x1, x2, conv_w):
    if USE_BF16:
        import ml_dtypes
        x1r = np.asarray(x1, dtype=np.float32).reshape(B, C, HW).astype(ml_dtypes.bfloat16)
        x2r = np.asarray(x2, dtype=np.float32).reshape(B, C, HW).astype(ml_dtypes.bfloat16)
    else:
        x1r = np.ascontiguousarray(x1, dtype=np.float32).reshape(B, C, HW)
        x2r = np.ascontiguousarray(x2, dtype=np.float32).reshape(B, C, HW)
    wcomb = np.empty((128, 3, C), np.float32)
    wcomb[:, 0:2, :] = conv_w[:, :C].T.reshape(2, 128, C).transpose(1, 0, 2)
    wcomb[:, 2, :] = conv_w[:, C] / C
    return [
        {
            "x1": x1r[c * B_LOC:(c + 1) * B_LOC],
            "x2": x2r[c * B_LOC:(c + 1) * B_LOC],
            "wc": wcomb,
        }
        for c in range(N_CORES)
    ]


def _postprocess(results):
    out = np.empty((B, C), np.float32)
    for c in range(N_CORES):
        rs = results[c]["rs"]
        r = rs[:, 0, :2 * ITEMS].copy()
        s = rs[:, 1, :2 * ITEMS].copy()
        # fold any partial-accumulator columns back in (zero when unused)
        for m in range(2):
            for j in range(3):
                r[:, 2 * (ITEMS - 1) + m] += rs[:, 0, 2 * ITEMS + 3 * m + j]
                s[:, 2 * (ITEMS - 1) + m] += rs[:, 1, 2 * ITEMS + 3 * m + j]
        # r[o, 2*itm + m] -> pool[itm, m*128 + o]
        pool_rs = (r / (s + 1e-8)).reshape(128, ITEMS, 2)
        pool = np.transpose(pool_rs, (1, 2, 0)).reshape(ITEMS, C)
        out[c * B_LOC:(c + 1) * B_LOC] = pool[:B_LOC] + pool[B_LOC:]
    return out


def _run(x1, x2, conv_w, **bass_kwargs):
    nc = _get_nc()
    in_maps = _make_in_maps(x1, x2, conv_w)
    res = run_bass_kernel_spmd(nc, in_maps, list(range(N_CORES)), **bass_kwargs)
    return _postprocess(res.results), res


def kernel(x1, x2, conv_w, ca_w1=None, ca_b1=None, ca_w2=None, ca_b2=None):
    out, _ = _run(x1, x2, conv_w)
    return out


# revision 47
# speedup vs baseline: 1.0034x; 1.0034x over previous
"""Trainium2 Bass kernel for nn_CPDFP_25701084299789 (pooling).

Reference math (B=64, C=256, H=W=32), per branch x in {x1, x2}:
    center  = x[:, :, 16, 16]                               (B, C)
    dot     = sum_c(x * center) / C                         (B, 1, H, W)
    attn    = sigmoid(conv_w @ concat([x, dot], ch))        (B, C, H, W)
    pool    = sum_hw(x * attn) / (sum_hw(attn) + 1e-8)      (B, C)
output = pool(x1) + pool(x2)   # the caLayer softmax is over a size-1
                               # axis == 1.0, so it contributes nothing.

Key algebraic simplification: the concat+1x1-conv is a rank-1 weight
update, Y_b = (W[:, :C] + w_last (outer) center_b / C) @ X_b, so no
extra matmuls for the `dot` channel are needed.

Sharding: data-parallel over batch across 8 cores; each core handles
8 batch items x 2 branches = 16 independent (256, 1024) items.
Params (conv_w-derived) replicated.
"""

import os
import threading
from contextlib import ExitStack

import numpy as np

import concourse.bacc as bacc
import concourse.mybir as mybir
import concourse.tile as tile
from concourse.bass_utils import run_bass_kernel_spmd

N_CORES = 8
B, C, HW = 64, 256, 1024          # batch, channels, H*W
B_LOC = B // N_CORES              # batch items per core
ITEMS = 2 * B_LOC                 # branch-items per core (x1 + x2)
CENTER = 16 * 32 + 16             # flat index of (h//2, w//2)
FP = mybir.dt.float32
FPR = mybir.dt.float32r           # same bits; PE runs 4x faster than fp32
BF = mybir.dt.bfloat16

# bf16 x-path: halves HBM traffic at the cost of ~bf16-level accuracy
# (rel err ~2e-3 vs f32r's ~1e-4) for only a ~10% modeled speedup (ACT/DVE
# see no bf16 gain on this op mix). Off by default.
USE_BF16 = os.environ.get("KERNEL_BF16", "0") == "1"
XDT = BF if USE_BF16 else FPR

_build_lock = threading.Lock()
_cached_nc = None


def _build():
    nc = bacc.Bacc()

    xdram_dt = BF if USE_BF16 else FP
    x1 = nc.dram_tensor("x1", [B_LOC, C, HW], xdram_dt, kind="ExternalInput")
    x2 = nc.dram_tensor("x2", [B_LOC, C, HW], xdram_dt, kind="ExternalInput")
    # wc[c, kh, o] = conv_w[o, kh*128 + c] for kh<2 (transposed lhsT layout);
    # wc[p, 2, o] = conv_w[o, C] / C (broadcast across partitions).
    wc = nc.dram_tensor("wc", [128, 3, C], FP, kind="ExternalInput")
    # rs[o_part, 0, 2*itm + m] = r = sum_hw(x*attn); rs[o_part, 1, ...] = s =
    # sum_hw(attn). Cols >= 2*ITEMS are spare partial-accumulator columns
    # (zeroed; _postprocess adds them unconditionally).
    NCOL = 2 * ITEMS + 6
    rs_out = nc.dram_tensor("rs", [128, 2, NCOL], FP, kind="ExternalOutput")

    with tile.TileContext(nc) as tc, ExitStack() as ctx:
        singles = ctx.enter_context(tc.tile_pool(name="singles", bufs=1))
        xpool = ctx.enter_context(tc.tile_pool(name="xp", bufs=10))
        wpool = ctx.enter_context(tc.tile_pool(name="wp", bufs=6))
        apool = ctx.enter_context(tc.tile_pool(name="ap", bufs=6))
        psum = ctx.enter_context(tc.tile_pool(name="ps", bufs=4, space="PSUM"))

        wc_sb = singles.tile([128, 3, C], FP)
        nc.sync.dma_start(out=wc_sb, in_=wc[:, :, :])
        rs_sb = singles.tile([128, 2, NCOL], FP)
        r_sb = rs_sb[:, 0, :]
        s_sb = rs_sb[:, 1, :]
        nc.vector.memset(rs_sb[:, :, 2 * ITEMS:], 0.0)
        # Absorb the weight-DMA wait into the consuming engines' vector
        # clocks up front, so per-item instructions don't each carry an
        # extra sync wait (walrus rejects ops with too many waits).
        absorb = singles.tile([128, 2], FP)
        nc.vector.tensor_copy(out=absorb[:, 0:1], in_=wc_sb[:, 0, 0:1])
        nc.gpsimd.tensor_copy(out=absorb[:, 1:2], in_=wc_sb[:, 0, 0:1])

        ADT = XDT if USE_BF16 else FP

        def make_weff(cen_aps):
            # weff[c, o] = wt[c, o] + x[c, center] * wlast[o]/C, on GpSimd
            # (Pool) — otherwise idle — keeping DVE for the r-reduce. Pool
            # rejects the fused STT op, so two steps.
            weffs = []
            for kh in range(2):
                if USE_BF16:
                    # tensor_scalar requires an fp32 scalar; upconvert first
                    cen = wpool.tile([128, 1], FP, tag="cen")
                    nc.gpsimd.tensor_copy(out=cen, in_=cen_aps[kh])
                    cen_ap = cen
                else:
                    cen_ap = cen_aps[kh].bitcast(FP)
                delta = wpool.tile([128, C], FP, tag="delta")
                nc.gpsimd.tensor_scalar_mul(delta, wc_sb[:, 2, :], cen_ap)
                weff = wpool.tile([128, C], XDT, tag="weff")
                nc.gpsimd.tensor_tensor(
                    out=weff, in0=delta, in1=wc_sb[:, kh, :],
                    op=mybir.AluOpType.add,
                )
                weffs.append(weff)
            return weffs

        def sig_and_reduce(y_ap, x_ap, a_ap, prod_ap, rcol):
            nc.scalar.activation(
                out=a_ap,
                in_=y_ap,
                func=mybir.ActivationFunctionType.Sigmoid,
                accum_out=s_sb[:, rcol:rcol + 1],
            )
            # Fused multiply + free-axis reduce: r = sum_hw(a * x).
            # (InstTensorTensorReduce miscompiles on this stack; the
            # TensorScalarPtr form with accum_out works.)
            nc.vector.scalar_tensor_tensor(
                out=prod_ap,
                in0=a_ap,
                scalar=1.0,
                in1=x_ap,
                op0=mybir.AluOpType.bypass,
                op1=mybir.AluOpType.mult,
                accum_out=r_sb[:, rcol:rcol + 1],
            )

        for itm in range(ITEMS - 1):
            src = x1 if itm < B_LOC else x2
            bi = itm % B_LOC

            xs = []
            for kh in range(2):
                xt = xpool.tile([128, HW], XDT, tag="x")
                din = src[bi, kh * 128:(kh + 1) * 128, :]
                nc.sync.dma_start(out=xt, in_=din if USE_BF16 else din.bitcast(FPR))
                xs.append(xt)

            weffs = make_weff([x[:, CENTER:CENTER + 1] for x in xs])

            for m in range(2):  # output-channel halves
                y = psum.tile([128, HW], FP, tag="y")
                for n, kh in [(0, 0), (1, 0), (0, 1), (1, 1)]:  # kh-outer: LDW reuse
                    nc.tensor.matmul(
                        out=y[:, n * 512:(n + 1) * 512],
                        lhsT=weffs[kh][:, m * 128:(m + 1) * 128],
                        rhs=xs[kh][:, n * 512:(n + 1) * 512],
                        start=(kh == 0),
                        stop=(kh == 1),
                    )
                a = apool.tile([128, HW], ADT, tag="a")
                prod = apool.tile([128, HW], ADT, tag="prod")
                sig_and_reduce(y, xs[m], a, prod, 2 * itm + m)

        # ---- Final item: its x lands last, so its compute chain is the
        # kernel's tail. Shorten it by (a) loading kh0 in full ahead of the
        # split kh1 so the kh0 matmuls run while kh1 streams, (b) splitting
        # kh1 into two half-tiles (hi half first — it holds the center
        # column, unblocking weff), and (c) giving each (m, n) quadrant its
        # own 1-bank PSUM tile + partial r/s column, so every sigmoid/reduce
        # fires as soon as ITS two matmuls finish.
        itm = ITEMS - 1
        src, bi = x2, B_LOC - 1

        x0 = xpool.tile([128, HW], XDT, tag="x")
        d0 = src[bi, 0:128, :]
        nc.sync.dma_start(out=x0, in_=d0 if USE_BF16 else d0.bitcast(FPR))
        xh = {}
        for n in (1, 0):  # hi half first: contains the center column
            xt = xpool.tile([128, 512], XDT, tag="xl")
            dn = src[bi, 128:256, n * 512:(n + 1) * 512]
            nc.sync.dma_start(out=xt, in_=dn if USE_BF16 else dn.bitcast(FPR))
            xh[n] = xt

        weffs = make_weff([x0[:, CENTER:CENTER + 1],
                           xh[1][:, CENTER - 512:CENTER - 511]])

        ys = {}
        for m in range(2):
            for n in range(2):
                ys[(m, n)] = psum.tile([128, 512], FP, tag="y",
                                       name=f"ylast_{m}_{n}")
                nc.tensor.matmul(
                    out=ys[(m, n)],
                    lhsT=weffs[0][:, m * 128:(m + 1) * 128],
                    rhs=x0[:, n * 512:(n + 1) * 512],
                    start=True,
                    stop=False,
                )
        for n in (1, 0):
            for m in range(2):
                nc.tensor.matmul(
                    out=ys[(m, n)],
                    lhsT=weffs[1][:, m * 128:(m + 1) * 128],
                    rhs=xh[n],
                    start=False,
                    stop=True,
                )
        for n in (1, 0):
            for m in range(2):
                # n=0 accumulates into the item's base column, n=1 into its
                # partial column (folded back on the host)
                rcol = 2 * itm + m if n == 0 else 2 * ITEMS + 3 * m
                xin = x0[:, n * 512:(n + 1) * 512] if m == 0 else xh[n]
                ac = apool.tile([128, 512], ADT, tag="a")
                pc = apool.tile([128, 512], ADT, tag="prod")
                sig_and_reduce(ys[(m, n)], xin, ac, pc, rcol)

        nc.sync.dma_start(out=rs_out[:, :, :], in_=rs_sb)

    nc.finalize()
    return nc


def _get_nc():
    global _cached_nc
    with _build_lock:
        if _cached_nc is None:
            _cached_nc = _build()
    return _cached_nc


def _make_in_maps(x1, x2, conv_w):
    conv_w = np.asarray(conv_w, dtype=np.float32)
    if USE_BF16:
        import ml_dtypes
        x1r = np.asarray(x1, dtype=np.float32).reshape(B, C, HW).astype(ml_dtypes.bfloat16)
        x2r = np.asarray(x2, dtype=np.float32).reshape(B, C, HW).astype(ml_dtypes.bfloat16)
    else:
        x1r = np.ascontiguousarray(x1, dtype=np.float32).reshape(B, C, HW)
        x2r = np.ascontiguousarray(x2, dtype=np.float32).reshape(B, C, HW)
    wcomb = np.empty((128, 3, C), np.float32)
    wcomb[:, 0:2, :] = conv_w[:, :C].T.reshape(2, 128, C).transpose(1, 0, 2)
    wcomb[:, 2, :] = conv_w[:, C] / C
    return [
        {
            "x1": x1r[c * B_LOC:(c + 1) * B_LOC],
            "x2": x2r[c * B_LOC:(c + 1) * B_LOC],
            "wc": wcomb,
        }
        for c in range(N_CORES)
    ]


def _postprocess(results):
    out = np.empty((B, C), np.float32)
    for c in range(N_CORES):
        rs = results[c]["rs"]
        r = rs[:, 0, :2 * ITEMS].copy()
        s = rs[:, 1, :2 * ITEMS].copy()
        # fold any partial-accumulator columns back in (zero when unused)
        for m in range(2):
            for j in range(3):
                r[:, 2 * (ITEMS - 1) + m] += rs[:, 0, 2 * ITEMS + 3 * m + j]
                s[:, 2 * (ITEMS - 1) + m] += rs[:, 1, 2 * ITEMS + 3 * m + j]
        # r[o, 2*itm + m] -> pool[itm, m*128 + o]
        pool_rs = (r / (s + 1e-8)).reshape(128, ITEMS, 2)
        pool = np.transpose(pool_rs, (1, 2, 0)).reshape(ITEMS, C)
        out[c * B_LOC:(c + 1) * B_LOC] = pool[:B_LOC] + pool[B_LOC:]
    return out


def _run(x1, x2, conv_w, **bass_kwargs):
    nc = _get_nc()
    in_maps = _make_in_maps(x1, x2, conv_w)
    res = run_bass_kernel_spmd(nc, in_maps, list(range(N_CORES)), **bass_kwargs)
    return _postprocess(res.results), res


def kernel(x1, x2, conv_w, ca_w1=None, ca_b1=None, ca_w2=None, ca_b2=None):
    out, _ = _run(x1, x2, conv_w)
    return out
